# revision 2
# baseline (speedup 1.0000x reference)
"""Distributed attention kernel for 8 trn2 NeuronCores.

Problem: B=2, N=2048, C=1024, H=16, D=64 attention with relative position
bias, qkv projection and output projection.

Sharding: head-parallel, 2 heads per core, both batches on every core.
Each core computes a partial output projection (its 2 heads' contribution
to all 1024 output channels); the host sums the 8 partials.

Device-side layouts (everything pre-transposed on host, bf16):
  - x is passed as xT [b, c, n]  (channels on partitions)
  - qk^T is computed transposed: scoresT[m, n] = k @ q.T per head, so the
    PV matmul needs no transposes (lhsT = v natural [m, d]).
  - softmax: exp(s + bias) = exp(s) * exp(bias); host precomputes
    exp(bias.T) in bf16, device multiplies (in place) after the ACT exp.
  - denominator: V is augmented with a ones column; PV psum row 64 then
    holds sum_m exp-scores; reciprocal + partition-broadcast multiply.

Toolchain constraint: compute instructions may carry only ONE semaphore
wait (DMAs: two).  Hence: all DMAs go through gpsimd SWDGE (single
semaphore, cumulative ticks), evictions per psum-slot stay on a single
engine, and a few tiny "observer" ops (ldweights / 1-elem copies) absorb
a second wait where structurally unavoidable.
"""

import sys

import numpy as np
import ml_dtypes

sys.path.insert(0, "/opt/trn_rl_repo")

B, N, C = 2, 2048, 1024
H, D = 16, 64
SCALE = D**-0.5
NCORES = 8
HPC = H // NCORES  # heads per core = 2

bf16 = ml_dtypes.bfloat16

_graph_cache = {}


def _fix_sync_waits(nc):
    """Walrus in this toolchain accepts at most ONE sync wait on compute
    instructions (two on DMAs).  Tile emits more.  Fix up the built graph:
      - drop waits on the instruction's own scheduled proc (in-order
        execution of a proc makes them always-satisfied),
      - hoist remaining excess waits onto standalone NoOps inserted just
        before the instruction on the same engine.
    """
    from concourse import mybir
    from concourse.tile_sem_assignment import PROC_NAME_TO_IDX

    idx_to_proc = {v: k for k, v in PROC_NAME_TO_IDX.items()}
    fixid = [0]
    for fn in nc.m.functions:
        for bb in fn.blocks:
            insts = list(bb.instructions)
            out = []
            changed = False
            for inst in insts:
                si = inst.sync_info
                tname = type(inst).__name__
                if not si or not si.on_wait:
                    out.append(inst)
                    continue
                own = idx_to_proc.get(inst.bass_scheduled_proc, None)
                waits = list(si.on_wait)
                if own is not None:
                    kept = [
                        w
                        for w in waits
                        if w.ant_name.rsplit("_", 1)[0] != own
                    ]
                else:
                    kept = waits
                limit = 1
                hoist = []
                if len(kept) > limit:
                    hoist = kept[: len(kept) - limit]
                    kept = kept[len(kept) - limit :]
                if len(kept) != len(waits) or hoist:
                    changed = True
                    for w in hoist:
                        fixid[0] += 1
                        nop = mybir.InstNoOp(
                            name=f"W-fix-{fixid[0]}",
                            ins=[],
                            outs=[],
                            engine=inst.engine,
                            bass_nofuse=True,
                            text_hint="wait-split",
                            sync_info=mybir.SyncInfo(on_wait=[w], on_update=[]),
                        )
                        out.append(nop)
                    si.on_wait = kept
                out.append(inst)
            if changed:
                bb.instructions = out


def _build_graph():
    import concourse.bass as bass
    import concourse.tile as tile
    from concourse import mybir

    EXP = mybir.ActivationFunctionType.Exp
    fp32 = mybir.dt.float32
    bfl = mybir.dt.bfloat16

    nc = bass.Bass()

    xt_d = nc.declare_dram_parameter("xt", [B, 8, 128, N], bfl, isOutput=False)
    wq_d = nc.declare_dram_parameter("wq", [128, 8, 128], bfl, isOutput=False)
    wk_d = nc.declare_dram_parameter("wk", [128, 8, 128], bfl, isOutput=False)
    wv_d = nc.declare_dram_parameter("wv", [128, 8, 128], bfl, isOutput=False)
    bq_d = nc.declare_dram_parameter("bq", [128, 1], fp32, isOutput=False)
    bv_d = nc.declare_dram_parameter("bv", [128, 128], fp32, isOutput=False)
    eb_d = nc.declare_dram_parameter(
        "expb", [2, 16, 128, HPC, 1024], bfl, isOutput=False
    )
    pw_d = nc.declare_dram_parameter("pw", [128, 1024], bfl, isOutput=False)
    out_d = nc.declare_dram_parameter("out", [B, 16, 128, C], bfl, isOutput=True)

    with tile.TileContext(nc) as tc:
        with (
            tc.tile_pool(name="weights", bufs=1) as wpool,
            tc.tile_pool(name="xt", bufs=1) as xpool,
            tc.tile_pool(name="qkvt", bufs=1) as qkpool,
            tc.tile_pool(name="eb", bufs=4) as ebpool,
            tc.tile_pool(name="es", bufs=16) as espool,
            tc.tile_pool(name="norm", bufs=2) as rpool,
            tc.tile_pool(name="outsb", bufs=4) as opool,
            tc.tile_pool(name="scratch", bufs=1) as spool,
            tc.tile_pool(name="psbig", bufs=2, space="PSUM") as psbig,
            tc.tile_pool(name="psacc", bufs=1, space="PSUM") as psacc,
            tc.tile_pool(name="dramsc", bufs=2, space="DRAM") as dpool,
        ):
            # ---- load weights (persistent), all on SWDGE ----
            wq = wpool.tile([128, 8, 128], bfl)
            wk = wpool.tile([128, 8, 128], bfl)
            wv = wpool.tile([128, 8, 128], bfl)
            pw = wpool.tile([128, 1024], bfl)
            bq = wpool.tile([128, 1], fp32)
            bv = wpool.tile([128, 128], fp32)
            nc.sync.dma_start(out=wq[:], in_=wq_d[:])
            nc.sync.dma_start(out=wk[:], in_=wk_d[:])
            nc.sync.dma_start(out=wv[:], in_=wv_d[:])
            nc.sync.dma_start(out=pw[:], in_=pw_d[:])
            nc.sync.dma_start(out=bq[:], in_=bq_d[:])
            nc.sync.dma_start(out=bv[:], in_=bv_d[:])
            # observers: DVE syncs on the bias DMAs once, so later evictions
            # carry only their PE wait (bq and bv land on different SWDGE sems)
            warm = spool.tile([128, 128], fp32)
            nc.vector.tensor_copy(out=warm[:, 0:1], in_=bq[:])
            nc.vector.tensor_copy(out=warm[:], in_=bv[:])
            act_scr = spool.tile([1, 8], bfl)
            dve_scr = spool.tile([1, 8], bfl)

            qt = [
                qkpool.tile([128, N], bfl, tag=f"qt{b}", name=f"qt{b}")
                for b in range(B)
            ]
            kt = [
                qkpool.tile([128, N], bfl, tag=f"kt{b}", name=f"kt{b}")
                for b in range(B)
            ]
            aot = [
                qkpool.tile([128, N], bfl, tag=f"aot{b}", name=f"aot{b}")
                for b in range(B)
            ]
            # v augmented with ones column: [m-part, mi, 65]
            vaug = [
                qkpool.tile([128, 16, 65], bfl, tag=f"vaug{u}", name=f"vaug{u}")
                for u in range(B * HPC)
            ]
            for u in range(B * HPC):
                nc.vector.memset(vaug[u][:, :, 64:65], 1.0)

            # ---- stage A: qkv for both batches ----
            for b in range(B):
                xts = []
                for ci in range(8):
                    xtile = xpool.tile([128, N], bfl, tag=f"xt{ci}", name=f"xt_{b}_{ci}")
                    nc.sync.dma_start(out=xtile[:], in_=xt_d[b, ci])
                    xts.append(xtile)
                if b > 0:
                    # PE observers: absorb the DMA waits for batch b's x tiles
                    # (one SWDGE sem each) so the first matmuls only wait on
                    # the psum slot's DVE reader
                    for ci in range(8):
                        nc.tensor.ldweights(weights=xts[ci][0:1, 0:1])
                # q^T and k^T: [128 j, N] = sum_c w[c, j] * xT[c, n]
                for ni in range(4):
                    nsl = slice(512 * ni, 512 * ni + 512)
                    psq = psbig.tile([128, 1024], fp32, tag="ps")
                    for ci in range(8):
                        nc.tensor.matmul(
                            psq[:, 0:512], lhsT=wq[:, ci, :], rhs=xts[ci][:, nsl],
                            start=(ci == 0), stop=(ci == 7),
                        )
                        nc.tensor.matmul(
                            psq[:, 512:1024], lhsT=wk[:, ci, :], rhs=xts[ci][:, nsl],
                            start=(ci == 0), stop=(ci == 7),
                        )
                    nc.vector.tensor_scalar_add(
                        out=qt[b][:, nsl], in0=psq[:, 0:512], scalar1=bq[:]
                    )
                    nc.vector.tensor_copy(out=kt[b][:, nsl], in_=psq[:, 512:1024])
                # v: [m-part, j2] per 128-row chunk
                for si in range(16):
                    msl = slice(128 * si, 128 * si + 128)
                    psv = psbig.tile([128, 1024], fp32, tag="ps")
                    for ci in range(8):
                        nc.tensor.matmul(
                            psv[:, 0:128], lhsT=xts[ci][:, msl], rhs=wv[:, ci, :],
                            start=(ci == 0), stop=(ci == 7),
                        )
                    for hi in range(HPC):
                        jsl = slice(64 * hi, 64 * hi + 64)
                        nc.vector.tensor_add(
                            out=vaug[b * HPC + hi][:, si, 0:64],
                            in0=psv[:, jsl], in1=bv[:, jsl],
                        )

            # ACT observer: sync ACT on DVE once before stage B (the scores
            # psum slots were last read by stage-A DVE evictions)
            nc.scalar.copy(out=act_scr[0:1, 0:1], in_=vaug[B * HPC - 1][0:1, 15, 0:1])

            # rec2[b]: per-element softmax reciprocal, broadcast across the
            # head dim partitions; applied to raw attention outputs once,
            # just before the projection
            rec2 = [
                qkpool.tile([128, N], fp32, tag=f"rec2{b}", name=f"rec2{b}")
                for b in range(B)
            ]

            # ---- stage B: attention, head pairs row/col-packed ----
            prev_es_last = None
            for b in range(B):
                for ni in range(2):
                    nsl = slice(1024 * ni, 1024 * ni + 1024)
                    if prev_es_last is not None:
                        # ACT observer: absorb an old DVE multiply tick so
                        # fresh es slots mostly carry only a PE wait
                        nc.scalar.copy(
                            out=act_scr[0:1, 1:2], in_=prev_es_last[0:1, 0:1]
                        )
                    pvA = psacc.tile([65, 1024], fp32, tag="pvA")
                    pvB = psacc.tile([65, 1024], fp32, tag="pvB")
                    for mi in range(16):
                        msl = slice(128 * mi, 128 * mi + 128)
                        eb2 = ebpool.tile([128, HPC, 1024], bfl)
                        nc.sync.dma_start(out=eb2[:], in_=eb_d[ni, mi])
                        # scores^T for both heads, row-packed: head A lives in
                        # K-rows 0:64, head B in 64:128 -> concurrent matmuls
                        psX = psbig.tile([128, 1024], fp32, tag="ps")
                        psY = psbig.tile([128, 1024], fp32, tag="ps")
                        for half in range(2):
                            fsl = slice(512 * half, 512 * half + 512)
                            qsl = slice(
                                1024 * ni + 512 * half, 1024 * ni + 512 * half + 512
                            )
                            nc.tensor.matmul(
                                psX[:, fsl], lhsT=kt[b][0:64, msl],
                                rhs=qt[b][0:64, qsl], start=True, stop=True,
                            )
                            nc.tensor.matmul(
                                psY[:, fsl], lhsT=kt[b][64:128, msl],
                                rhs=qt[b][64:128, qsl], start=True, stop=True,
                            )
                        esA = espool.tile([128, 1024], bfl)
                        nc.scalar.activation(out=esA[:], in_=psX[:], func=EXP)
                        esB = espool.tile([128, 1024], bfl)
                        nc.scalar.activation(out=esB[:], in_=psY[:], func=EXP)
                        # DVE observer: absorb the eb2 DMA wait
                        nc.vector.tensor_copy(out=dve_scr[0:1, 0:1], in_=eb2[0:1, 0, 0:1])
                        nc.vector.tensor_mul(out=esA[:], in0=esA[:], in1=eb2[:, 0, :])
                        nc.vector.tensor_mul(out=esB[:], in0=esB[:], in1=eb2[:, 1, :])
                        for half in range(2):
                            fsl = slice(512 * half, 512 * half + 512)
                            nc.tensor.matmul(
                                pvA[0:65, fsl], lhsT=vaug[b * HPC][:, mi, :],
                                rhs=esA[:, fsl],
                                start=(mi == 0), stop=(mi == 15),
                            )
                            nc.tensor.matmul(
                                pvB[0:65, fsl], lhsT=vaug[b * HPC + 1][:, mi, :],
                                rhs=esB[:, fsl],
                                start=(mi == 0), stop=(mi == 15),
                            )
                        prev_es_last = esB
                    # drain the PV psums quickly so the next (b, ni) block's PV
                    # can reuse the slot: evict raw outputs + the sums row.
                    # The reciprocal runs on a [128, 8] relayout (via a DRAM
                    # bounce) — a [1, 1024] single-partition reciprocal costs
                    # 6.5us of DVE and stalls the PE long enough to re-throttle
                    # the HAM clock gate.
                    for hi, pv in ((0, pvA), (1, pvB)):
                        hs = slice(64 * hi, 64 * hi + 64)
                        dn = rpool.tile([1, 1024], fp32, tag=f"dn{hi}")
                        if hi == 0:
                            nc.scalar.copy(out=dn[:], in_=pv[64:65, :])
                            nc.scalar.copy(out=aot[b][hs, nsl], in_=pv[0:64, :])
                        else:
                            nc.vector.tensor_copy(out=dn[:], in_=pv[64:65, :])
                            nc.vector.tensor_copy(
                                out=aot[b][hs, nsl], in_=pv[0:64, :]
                            )
                        # async chain, off the PE critical path:
                        # dn -> DRAM -> [128,8] -> reciprocal -> DRAM -> bcast
                        d1 = dpool.tile([1, 1024], fp32, tag=f"d1{hi}")
                        nc.sync.dma_start(out=d1[:], in_=dn[:])
                        dc = rpool.tile([128, 8], fp32, tag=f"dc{hi}")
                        d1_col = bass.AP(
                            tensor=d1.tensor,
                            offset=d1.offset,
                            ap=[[8, 128], [1, 8]],
                        )
                        nc.sync.dma_start(out=dc[:], in_=d1_col)
                        rcol = rpool.tile([128, 8], fp32, tag=f"rc{hi}")
                        nc.vector.reciprocal(out=rcol[:], in_=dc[:])
                        d2 = dpool.tile([1, 1024], fp32, tag=f"d2{hi}")
                        d2_col = bass.AP(
                            tensor=d2.tensor,
                            offset=d2.offset,
                            ap=[[8, 128], [1, 8]],
                        )
                        nc.sync.dma_start(out=d2_col, in_=rcol[:])
                        d2_bcast = bass.AP(
                            tensor=d2.tensor,
                            offset=d2.offset,
                            ap=[[0, 64]] + list(d2.ap),
                        )
                        nc.sync.dma_start(out=rec2[b][hs, nsl], in_=d2_bcast)

            # ---- stage C: normalize + output projection (partial) ----
            for b in range(B):
                # apply the deferred softmax normalization in one pass
                nc.vector.tensor_mul(out=aot[b][:], in0=aot[b][:], in1=rec2[b][:])
                # PE observer: absorb aot's fresh DVE tick at batch entry
                nc.tensor.ldweights(weights=aot[b][0:1, 0:1])
                for si in range(16):
                    msl = slice(128 * si, 128 * si + 128)
                    psp = psbig.tile([128, 1024], fp32, tag="ps")
                    for half in range(2):
                        fsl = slice(512 * half, 512 * half + 512)
                        nc.tensor.matmul(
                            psp[:, fsl], lhsT=aot[b][:, msl], rhs=pw[:, fsl],
                            start=True, stop=True,
                        )
                    ob = opool.tile([128, 1024], bfl)
                    # evictions alternate engines by si parity so each proj
                    # matmul needs only one wait (same-parity slot reuse).
                    # A tiny write first absorbs the out-DMA's WAR tick on
                    # the ob slot, so the eviction itself only waits on PE.
                    if si % 2 == 0:
                        nc.scalar.copy(out=ob[0:1, 0:1], in_=act_scr[0:1, 0:1])
                        nc.scalar.copy(out=ob[:], in_=psp[:])
                    else:
                        nc.vector.memset(ob[0:1, 0:1], 0.0)
                        nc.vector.tensor_copy(out=ob[:], in_=psp[:])
                    nc.sync.dma_start(out=out_d[b, si], in_=ob[:])

    _fix_sync_waits(nc)
    return nc


def _prep_inputs(x, rel_pos_bias, qkv_w, q_bias, v_bias):
    """Build the 8 per-core input maps (host-side shard + transpose + cast)."""
    x = np.asarray(x, dtype=np.float32)
    rel_pos_bias = np.asarray(rel_pos_bias, dtype=np.float32)
    qkv_w = np.asarray(qkv_w, dtype=np.float32)
    q_bias = np.asarray(q_bias, dtype=np.float32)
    v_bias = np.asarray(v_bias, dtype=np.float32)

    # xT: [b, c, n] -> [b, 8, 128, n]
    xt = np.ascontiguousarray(x.transpose(0, 2, 1)).reshape(B, 8, 128, N).astype(bf16)

    in_maps = []
    for c in range(NCORES):
        heads = [HPC * c + i for i in range(HPC)]
        jrows = np.concatenate([np.arange(64 * h, 64 * h + 64) for h in heads])

        def tile_w(rows, scale=1.0):
            wt = (scale * qkv_w[rows]).T.astype(bf16)  # [1024 c, 128 j]
            return np.ascontiguousarray(wt.reshape(8, 128, 128).transpose(1, 0, 2))

        wq = tile_w(jrows, SCALE)
        wk = tile_w(C + jrows)
        wv = tile_w(2 * C + jrows)
        bq = (SCALE * q_bias[jrows]).reshape(128, 1).astype(np.float32)
        bv = np.ascontiguousarray(
            np.broadcast_to(v_bias[jrows][None, :], (128, 128)).astype(np.float32)
        )
        # exp of transposed bias, tiled: [ni, mi, 128 m, hpc, 1024 n]
        ebt = np.exp(rel_pos_bias[heads].transpose(0, 2, 1))  # [hpc, m, n]
        ebt = np.ascontiguousarray(
            ebt.reshape(HPC, 16, 128, 2, 1024).transpose(3, 1, 2, 0, 4)
        ).astype(bf16)
        in_maps.append(
            {"xt": xt, "wq": wq, "wk": wk, "wv": wv, "bq": bq, "bv": bv, "expb": ebt}
        )
    return in_maps


def kernel(x, rel_pos_bias, qkv_w, q_bias, v_bias, proj_w, proj_b):
    from concourse.bass_utils import run_bass_kernel_spmd

    x = np.asarray(x, dtype=np.float32)
    proj_w = np.asarray(proj_w, dtype=np.float32)
    proj_b = np.asarray(proj_b, dtype=np.float32)

    if "nc" not in _graph_cache:
        _graph_cache["nc"] = _build_graph()
    nc = _graph_cache["nc"]

    in_maps = _prep_inputs(x, rel_pos_bias, qkv_w, q_bias, v_bias)
    for c in range(NCORES):
        heads = [HPC * c + i for i in range(HPC)]
        jrows = np.concatenate([np.arange(64 * h, 64 * h + 64) for h in heads])
        pw = np.ascontiguousarray(proj_w[:, jrows].T.astype(bf16))  # [128 j, 1024 e]
        in_maps[c]["pw"] = pw

    res = run_bass_kernel_spmd(nc, in_maps, core_ids=list(range(NCORES)))
    out = np.zeros((B, 16, 128, C), dtype=np.float32)
    for r in res.results:
        out += np.asarray(r["out"], dtype=np.float32)
    out = out.reshape(B, N, C) + proj_b[None, None, :]
    return out



# revision 13
# speedup vs baseline: 1.2485x; 1.2485x over previous
"""Distributed attention kernel for 8 trn2 NeuronCores.

Problem: B=2, N=2048, C=1024, H=16, D=64 attention with relative position
bias, qkv projection and output projection.

Sharding: head-parallel, 2 heads per core, both batches on every core.
Each core computes a partial output projection (its 2 heads' contribution
to all 1024 output channels); the host sums the 8 partials.

Device-side layouts (everything pre-transposed on host, bf16):
  - x is passed as xT [b, c, n]  (channels on partitions)
  - qk^T is computed transposed: scoresT[m, n] = k @ q.T per head, so the
    PV matmul needs no transposes (lhsT = v natural [m, d]).
  - softmax: exp(s + bias) = exp(s) * exp(bias); host precomputes
    exp(bias.T) in bf16, device multiplies after the ACT exp.
  - denominator: V is augmented with a ones column; PV psum row 64 then
    holds sum_m exp-scores; reciprocal (done on a [128, 8] relayout via a
    DRAM bounce — a [1, 1024] single-partition reciprocal costs 6.5us of
    DVE) + partition-broadcast multiply.

Scheduling notes (issue order == per-engine execution order):
  - stage B is software-pipelined: scores(mi+1) are issued BEFORE PV(mi),
    so the PE never waits out the exp+mul latency mid-iteration.
  - stage A (batch 1) and stage C (batch 0) matmul bursts are injected at
    stage-B block boundaries: they fill the ACT-idle windows AND re-warm
    the PE HAM clock gate after any drain stall.

Toolchain constraint: compute instructions may carry only ONE semaphore
wait (DMAs: two).  Hence: all DMAs go through gpsimd SWDGE (single
semaphore, cumulative ticks), evictions per psum-slot stay on a single
engine, and a few tiny "observer" ops (ldweights / 1-elem copies) absorb
a second wait where structurally unavoidable.  _fix_sync_waits hoists any
remaining excess waits onto NoOps.
"""

import sys

import numpy as np
import ml_dtypes

sys.path.insert(0, "/opt/trn_rl_repo")

B, N, C = 2, 2048, 1024
H, D = 16, 64
SCALE = D**-0.5
NCORES = 8
HPC = H // NCORES  # heads per core = 2

bf16 = ml_dtypes.bfloat16

_graph_cache = {}


def _fix_sync_waits(nc):
    """Walrus in this toolchain accepts at most ONE sync wait on compute
    instructions (two on DMAs).  Tile emits more.  Fix up the built graph:
      - drop waits on the instruction's own scheduled proc (in-order
        execution of a proc makes them always-satisfied),
      - hoist remaining excess waits onto standalone NoOps inserted just
        before the instruction on the same engine.
    """
    from concourse import mybir
    from concourse.tile_sem_assignment import PROC_NAME_TO_IDX

    idx_to_proc = {v: k for k, v in PROC_NAME_TO_IDX.items()}
    fixid = [0]
    for fn in nc.m.functions:
        for bb in fn.blocks:
            insts = list(bb.instructions)
            out = []
            changed = False
            for inst in insts:
                si = inst.sync_info
                if not si or not si.on_wait:
                    out.append(inst)
                    continue
                own = idx_to_proc.get(inst.bass_scheduled_proc, None)
                waits = list(si.on_wait)
                if own is not None:
                    kept = [
                        w
                        for w in waits
                        if w.ant_name.rsplit("_", 1)[0] != own
                    ]
                else:
                    kept = waits
                limit = 1
                hoist = []
                if len(kept) > limit:
                    hoist = kept[: len(kept) - limit]
                    kept = kept[len(kept) - limit :]
                if len(kept) != len(waits) or hoist:
                    changed = True
                    for w in hoist:
                        fixid[0] += 1
                        nop = mybir.InstNoOp(
                            name=f"W-fix-{fixid[0]}",
                            ins=[],
                            outs=[],
                            engine=inst.engine,
                            bass_nofuse=True,
                            text_hint="wait-split",
                            sync_info=mybir.SyncInfo(on_wait=[w], on_update=[]),
                        )
                        out.append(nop)
                    si.on_wait = kept
                out.append(inst)
            if changed:
                bb.instructions = out
    return nc


def _build_graph():
    import concourse.bass as bass
    import concourse.tile as tile
    from concourse import mybir

    EXP = mybir.ActivationFunctionType.Exp
    fp32 = mybir.dt.float32
    bfl = mybir.dt.bfloat16

    nc = bass.Bass()

    xt_d = nc.declare_dram_parameter("xt", [B, 8, 128, N], bfl, isOutput=False)
    wq_d = nc.declare_dram_parameter("wq", [128, 8, 128], bfl, isOutput=False)
    wk_d = nc.declare_dram_parameter("wk", [128, 8, 128], bfl, isOutput=False)
    wv_d = nc.declare_dram_parameter("wv", [128, 8, 128], bfl, isOutput=False)
    bq_d = nc.declare_dram_parameter("bq", [128, 1], fp32, isOutput=False)
    bv_d = nc.declare_dram_parameter("bv", [128, 128], fp32, isOutput=False)
    eb_d = nc.declare_dram_parameter(
        "expb", [2, 16, 128, HPC * 1024], bfl, isOutput=False
    )
    pw_d = nc.declare_dram_parameter("pw", [128, 1024], bfl, isOutput=False)
    out_d = nc.declare_dram_parameter("out", [B, 16, 128, C], bfl, isOutput=True)
    dbg = {}
    if _graph_cache.get("debug"):
        dbg["qt"] = nc.declare_dram_parameter("dbg_qt", [128, N], bfl, isOutput=True)
        dbg["kt"] = nc.declare_dram_parameter("dbg_kt", [128, N], bfl, isOutput=True)
        dbg["va"] = nc.declare_dram_parameter("dbg_va", [128, 16, 65], bfl, isOutput=True)
        dbg["aot"] = nc.declare_dram_parameter("dbg_aot", [128, N], bfl, isOutput=True)
        dbg["rec"] = nc.declare_dram_parameter("dbg_rec", [128, N], fp32, isOutput=True)
        dbg["es"] = nc.declare_dram_parameter("dbg_es", [128, 2048], bfl, isOutput=True)

    with tile.TileContext(nc) as tc:
        with (
            tc.tile_pool(name="weights", bufs=1) as wpool,
            tc.tile_pool(name="xt", bufs=1) as xpool,
            tc.tile_pool(name="qkvt", bufs=1) as qkpool,
            tc.tile_pool(name="eb", bufs=4) as ebpool,
            tc.tile_pool(name="es", bufs=6) as espool,
            tc.tile_pool(name="norm", bufs=2) as rpool,
            tc.tile_pool(name="outsb", bufs=4) as opool,
            tc.tile_pool(name="scratch", bufs=1) as spool,
            tc.tile_pool(name="psbig", bufs=2, space="PSUM") as psbig,
            tc.tile_pool(name="psacc", bufs=1, space="PSUM") as psacc,
            tc.tile_pool(name="dramsc", bufs=2, space="DRAM") as dpool,
        ):
            # ---- persistent weights, all on SWDGE ----
            wq = wpool.tile([128, 8, 128], bfl)
            wk = wpool.tile([128, 8, 128], bfl)
            wv = wpool.tile([128, 8, 128], bfl)
            pw = wpool.tile([128, 1024], bfl)
            bq = wpool.tile([128, 1], fp32)
            bv = wpool.tile([128, 128], fp32)
            nc.sync.dma_start(out=wq[:], in_=wq_d[:])
            nc.sync.dma_start(out=wk[:], in_=wk_d[:])
            nc.sync.dma_start(out=wv[:], in_=wv_d[:])
            nc.sync.dma_start(out=pw[:], in_=pw_d[:])
            nc.sync.dma_start(out=bq[:], in_=bq_d[:])
            nc.sync.dma_start(out=bv[:], in_=bv_d[:])
            # observers: DVE syncs on the bias DMAs once, so later evictions
            # carry only their PE wait
            warm = spool.tile([128, 128], fp32)
            nc.vector.tensor_copy(out=warm[:, 0:1], in_=bq[:])
            nc.vector.tensor_copy(out=warm[:], in_=bv[:])
            act_scr = spool.tile([1, 8], bfl)
            dve_scr = spool.tile([1, 8], bfl)

            # ---- all x tiles for BOTH batches, prefetched up front ----
            xts = {}
            for b in range(B):
                for ci in range(8):
                    xtile = xpool.tile(
                        [128, N], bfl, tag=f"xt{b}_{ci}", name=f"xt_{b}_{ci}"
                    )
                    nc.sync.dma_start(out=xtile[:], in_=xt_d[b, ci])
                    xts[(b, ci)] = xtile

            qt = [
                qkpool.tile([128, N], bfl, tag=f"qt{b}", name=f"qt{b}")
                for b in range(B)
            ]
            kt = [
                qkpool.tile([128, N], bfl, tag=f"kt{b}", name=f"kt{b}")
                for b in range(B)
            ]
            aot = [
                qkpool.tile([128, N], bfl, tag=f"aot{b}", name=f"aot{b}")
                for b in range(B)
            ]
            # v augmented with ones column: [m-part, mi, 65]
            vaug = [
                qkpool.tile([128, 16, 65], bfl, tag=f"vaug{u}", name=f"vaug{u}")
                for u in range(B * HPC)
            ]
            for u in range(B * HPC):
                nc.vector.memset(vaug[u][:, :, 64:65], 1.0)

            # softmax reciprocal, bf16, broadcast across head-dim partitions;
            # applied to raw attention outputs once, just before projection
            rec2 = [
                qkpool.tile([128, N], fp32, tag=f"rec2{b}", name=f"rec2{b}")
                for b in range(B)
            ]

            # ---- stage helpers (call order == engine issue order) ----

            def qk_tile(b, ni):
                """q^T,k^T for one 512-col n-chunk: [128 j, 512] each."""
                nsl = slice(512 * ni, 512 * ni + 512)
                psq = psbig.tile([128, 1024], fp32, tag="ps")
                for ci in range(8):
                    nc.tensor.matmul(
                        psq[:, 0:512], lhsT=wq[:, ci, :], rhs=xts[(b, ci)][:, nsl],
                        start=(ci == 0), stop=(ci == 7),
                    )
                    nc.tensor.matmul(
                        psq[:, 512:1024], lhsT=wk[:, ci, :], rhs=xts[(b, ci)][:, nsl],
                        start=(ci == 0), stop=(ci == 7),
                    )
                nc.vector.tensor_scalar_add(
                    out=qt[b][:, nsl], in0=psq[:, 0:512], scalar1=bq[:]
                )
                nc.vector.tensor_copy(out=kt[b][:, nsl], in_=psq[:, 512:1024])

            def v_chunk(b, si):
                """v rows [128 m] for one si chunk, both heads."""
                msl = slice(128 * si, 128 * si + 128)
                psv = psbig.tile([128, 1024], fp32, tag="ps")
                for ci in range(8):
                    nc.tensor.matmul(
                        psv[:, 0:128], lhsT=xts[(b, ci)][:, msl], rhs=wv[:, ci, :],
                        start=(ci == 0), stop=(ci == 7),
                    )
                for hi in range(HPC):
                    jsl = slice(64 * hi, 64 * hi + 64)
                    nc.vector.tensor_add(
                        out=vaug[b * HPC + hi][:, si, 0:64],
                        in0=psv[:, jsl], in1=bv[:, jsl],
                    )

            state = {"prev_es": None}

            def b_block(b, ni):
                """16 software-pipelined attention iterations + drain."""
                nsl = slice(1024 * ni, 1024 * ni + 1024)
                pvA = psacc.tile([65, 1024], fp32, tag="pvA")
                pvB = psacc.tile([65, 1024], fp32, tag="pvB")

                def scores(mi):
                    msl = slice(128 * mi, 128 * mi + 128)
                    ps = psbig.tile([128, 1024], fp32, tag="ps", name=f"psX_{b}_{ni}_{mi}")
                    ps2 = psbig.tile([128, 1024], fp32, tag="ps", name=f"psY_{b}_{ni}_{mi}")
                    for half in range(2):
                        fsl = slice(512 * half, 512 * half + 512)
                        qsl = slice(
                            1024 * ni + 512 * half, 1024 * ni + 512 * half + 512
                        )
                        nc.tensor.matmul(
                            ps[:, fsl], lhsT=kt[b][0:64, msl],
                            rhs=qt[b][0:64, qsl], start=True, stop=True,
                        )
                        nc.tensor.matmul(
                            ps2[:, fsl], lhsT=kt[b][64:128, msl],
                            rhs=qt[b][64:128, qsl], start=True, stop=True,
                        )
                    return ps, ps2

                def pv(mi, es2):
                    for half in range(2):
                        fsl = slice(512 * half, 512 * half + 512)
                        f2 = slice(1024 + 512 * half, 1024 + 512 * half + 512)
                        nc.tensor.matmul(
                            pvA[0:65, fsl], lhsT=vaug[b * HPC][:, mi, :],
                            rhs=es2[:, fsl],
                            start=(mi == 0), stop=(mi == 15),
                        )
                        nc.tensor.matmul(
                            pvB[0:65, fsl], lhsT=vaug[b * HPC + 1][:, mi, :],
                            rhs=es2[:, f2],
                            start=(mi == 0), stop=(mi == 15),
                        )

                pending = None  # (mi, es2) awaiting PV issue
                for mi in range(16):
                    eb2 = ebpool.tile([128, HPC * 1024], bfl)
                    nc.sync.dma_start(out=eb2[:], in_=eb_d[ni, mi])
                    if state["prev_es"] is not None:
                        # ACT observer: absorb an old DVE multiply tick so
                        # fresh es slots mostly carry only a PE wait
                        nc.scalar.copy(
                            out=act_scr[0:1, 1:2], in_=state["prev_es"][0:1, 0:1]
                        )
                    psX, psY = scores(mi)
                    if pending is not None:
                        pv(*pending)
                    # es2 holds both heads side by side: [128, 2048]
                    es2 = espool.tile([128, 2 * 1024], bfl)
                    nc.scalar.activation(out=es2[:, 0:1024], in_=psX[:], func=EXP)
                    nc.scalar.activation(out=es2[:, 1024:2048], in_=psY[:], func=EXP)
                    # DVE observer: absorb the eb2 DMA wait
                    nc.vector.tensor_copy(out=dve_scr[0:1, 0:1], in_=eb2[0:1, 0:1])
                    nc.vector.tensor_mul(out=es2[:], in0=es2[:], in1=eb2[:])
                    if dbg and b == 0 and ni == 0 and mi == 0:
                        nc.sync.dma_start(out=dbg["es"][:], in_=es2[:])
                    pending = (mi, es2)
                    state["prev_es"] = es2
                pv(*pending)

                # drain the PV psums: raw outputs + the sums row, then the
                # reciprocal chain on a [128, 8] relayout via DRAM bounce
                for hi, pvt in ((0, pvA), (1, pvB)):
                    hs = slice(64 * hi, 64 * hi + 64)
                    dn = rpool.tile([1, 1024], fp32, tag=f"dn{hi}")
                    if hi == 0:
                        nc.scalar.copy(out=dn[:], in_=pvt[64:65, :])
                        nc.scalar.copy(out=aot[b][hs, nsl], in_=pvt[0:64, :])
                    else:
                        nc.vector.tensor_copy(out=dn[:], in_=pvt[64:65, :])
                        nc.vector.tensor_copy(
                            out=aot[b][hs, nsl], in_=pvt[0:64, :]
                        )
                    d1 = dpool.tile([1, 1024], fp32, tag=f"d1{hi}")
                    nc.sync.dma_start(out=d1[:], in_=dn[:])
                    dc = rpool.tile([128, 8], fp32, tag=f"dc{hi}")
                    d1_col = bass.AP(
                        tensor=d1.tensor, offset=d1.offset,
                        ap=[[8, 128], [1, 8]],
                    )
                    nc.sync.dma_start(out=dc[:], in_=d1_col)
                    rcol = rpool.tile([128, 8], fp32, tag=f"rc{hi}")
                    nc.vector.reciprocal(out=rcol[:], in_=dc[:])
                    d2 = dpool.tile([1, 1024], fp32, tag=f"d2{hi}")
                    d2_col = bass.AP(
                        tensor=d2.tensor, offset=d2.offset,
                        ap=[[8, 128], [1, 8]],
                    )
                    nc.sync.dma_start(out=d2_col, in_=rcol[:])
                    d2_bcast = bass.AP(
                        tensor=d2.tensor, offset=d2.offset,
                        ap=[[0, 64]] + list(d2.ap),
                    )
                    nc.sync.dma_start(out=rec2[b][hs, nsl], in_=d2_bcast)

            def c_norm(b):
                """apply the deferred softmax normalization in one pass"""
                if dbg and b == 0:
                    nc.sync.dma_start(out=dbg["qt"][:], in_=qt[0][:])
                    nc.sync.dma_start(out=dbg["kt"][:], in_=kt[0][:])
                    nc.sync.dma_start(out=dbg["va"][:], in_=vaug[0][:])
                    nc.sync.dma_start(out=dbg["rec"][:], in_=rec2[0][:])
                nc.vector.tensor_mul(out=aot[b][:], in0=aot[b][:], in1=rec2[b][:])
                if dbg and b == 0:
                    nc.sync.dma_start(out=dbg["aot"][:], in_=aot[0][:])
                # PE observer: absorb aot's fresh DVE tick
                nc.tensor.ldweights(weights=aot[b][0:1, 0:1])

            def c_chunk(b, si):
                msl = slice(128 * si, 128 * si + 128)
                psp = psbig.tile([128, 1024], fp32, tag="ps")
                for half in range(2):
                    fsl = slice(512 * half, 512 * half + 512)
                    nc.tensor.matmul(
                        psp[:, fsl], lhsT=aot[b][:, msl], rhs=pw[:, fsl],
                        start=True, stop=True,
                    )
                ob = opool.tile([128, 1024], bfl)
                # evictions alternate engines by si parity; a tiny write
                # first absorbs the out-DMA's WAR tick on the ob slot
                if si % 2 == 0:
                    nc.scalar.copy(out=ob[0:1, 0:1], in_=act_scr[0:1, 0:1])
                    nc.scalar.copy(out=ob[:], in_=psp[:])
                else:
                    nc.vector.memset(ob[0:1, 0:1], 0.0)
                    nc.vector.tensor_copy(out=ob[:], in_=psp[:])
                nc.sync.dma_start(out=out_d[b, si], in_=ob[:])

            # ---- schedule ----
            for ni in range(4):
                qk_tile(0, ni)
            for si in range(16):
                v_chunk(0, si)
            # ACT observer: sync ACT on DVE once before first exps
            nc.scalar.copy(
                out=act_scr[0:1, 0:1], in_=vaug[HPC - 1][0:1, 15, 0:1]
            )
            b_block(0, 0)
            # dense PE burst at the boundary: re-warms HAM, hides stage A(b1)
            qk_tile(1, 0)
            qk_tile(1, 1)
            b_block(0, 1)
            qk_tile(1, 2)
            qk_tile(1, 3)
            for si in range(16):
                v_chunk(1, si)
            b_block(1, 0)
            # stage C(b0) burst: ACT/DVE evictions fill the boundary window
            c_norm(0)
            for si in range(4):
                c_chunk(0, si)
            b_block(1, 1)
            for si in range(4, 16):
                c_chunk(0, si)
            c_norm(1)
            for si in range(16):
                c_chunk(1, si)

    _fix_sync_waits(nc)
    return nc


def _prep_inputs(x, rel_pos_bias, qkv_w, q_bias, v_bias):
    """Build the 8 per-core input maps (host-side shard + transpose + cast)."""
    x = np.asarray(x, dtype=np.float32)
    rel_pos_bias = np.asarray(rel_pos_bias, dtype=np.float32)
    qkv_w = np.asarray(qkv_w, dtype=np.float32)
    q_bias = np.asarray(q_bias, dtype=np.float32)
    v_bias = np.asarray(v_bias, dtype=np.float32)

    # xT: [b, c, n] -> [b, 8, 128, n]
    xt = np.ascontiguousarray(x.transpose(0, 2, 1)).reshape(B, 8, 128, N).astype(bf16)

    in_maps = []
    for c in range(NCORES):
        heads = [HPC * c + i for i in range(HPC)]
        jrows = np.concatenate([np.arange(64 * h, 64 * h + 64) for h in heads])

        def tile_w(rows, scale=1.0):
            wt = (scale * qkv_w[rows]).T.astype(bf16)  # [1024 c, 128 j]
            return np.ascontiguousarray(wt.reshape(8, 128, 128).transpose(1, 0, 2))

        wq = tile_w(jrows, SCALE)
        wk = tile_w(C + jrows)
        wv = tile_w(2 * C + jrows)
        bq = (SCALE * q_bias[jrows]).reshape(128, 1).astype(np.float32)
        bv = np.ascontiguousarray(
            np.broadcast_to(v_bias[jrows][None, :], (128, 128)).astype(np.float32)
        )
        # exp of transposed bias, tiled: [ni, mi, 128 m, hpc, 1024 n]
        ebt = np.exp(rel_pos_bias[heads].transpose(0, 2, 1))  # [hpc, m, n]
        ebt = np.ascontiguousarray(
            ebt.reshape(HPC, 16, 128, 2, 1024).transpose(3, 1, 2, 0, 4)
        ).astype(bf16).reshape(2, 16, 128, HPC * 1024)
        in_maps.append(
            {"xt": xt, "wq": wq, "wk": wk, "wv": wv, "bq": bq, "bv": bv, "expb": ebt}
        )
    return in_maps


def kernel(x, rel_pos_bias, qkv_w, q_bias, v_bias, proj_w, proj_b):
    from concourse.bass_utils import run_bass_kernel_spmd

    x = np.asarray(x, dtype=np.float32)
    proj_w = np.asarray(proj_w, dtype=np.float32)
    proj_b = np.asarray(proj_b, dtype=np.float32)

    if "nc" not in _graph_cache:
        _graph_cache["nc"] = _build_graph()
    nc = _graph_cache["nc"]

    in_maps = _prep_inputs(x, rel_pos_bias, qkv_w, q_bias, v_bias)
    for c in range(NCORES):
        heads = [HPC * c + i for i in range(HPC)]
        jrows = np.concatenate([np.arange(64 * h, 64 * h + 64) for h in heads])
        pw = np.ascontiguousarray(proj_w[:, jrows].T.astype(bf16))  # [128 j, 1024 e]
        in_maps[c]["pw"] = pw

    res = run_bass_kernel_spmd(nc, in_maps, core_ids=list(range(NCORES)))
    out = np.zeros((B, 16, 128, C), dtype=np.float32)
    for r in res.results:
        out += np.asarray(r["out"], dtype=np.float32)
    out = out.reshape(B, N, C) + proj_b[None, None, :]
    return out


# revision 17
# speedup vs baseline: 1.4840x; 1.1886x over previous
"""Distributed attention kernel for 8 trn2 NeuronCores.

Problem: B=2, N=2048, C=1024, H=16, D=64 attention with relative position
bias, qkv projection and output projection.

Sharding: head-parallel, 2 heads per core, both batches on every core.
Each core computes a partial output projection (its 2 heads' contribution
to all 1024 output channels); the host sums the 8 partials.

Device-side layouts (everything pre-transposed on host, bf16):
  - x is passed as xT [b, c, n]  (channels on partitions)
  - qk^T is computed transposed: scoresT[m, n] = k @ q.T per head, so the
    PV matmul needs no transposes (lhsT = v natural [m, d]).
  - softmax: exp(s + bias) = exp(s) * exp(bias); host precomputes
    exp(bias.T) in bf16, device multiplies after the ACT exp.
  - denominator: V is augmented with a ones column; PV psum row 64 then
    holds sum_m exp-scores; reciprocal (done on a [128, 8] relayout via a
    DRAM bounce — a [1, 1024] single-partition reciprocal costs 6.5us of
    DVE) + partition-broadcast multiply.

Scheduling notes (issue order == per-engine execution order):
  - stage B is software-pipelined: scores(mi+1) are issued BEFORE PV(mi),
    so the PE never waits out the exp+mul latency mid-iteration.
  - stage A (batch 1) and stage C (batch 0) matmul bursts are injected at
    stage-B block boundaries: they fill the ACT-idle windows AND re-warm
    the PE HAM clock gate after any drain stall.

Toolchain constraint: compute instructions may carry only ONE semaphore
wait (DMAs: two).  Hence: all DMAs go through gpsimd SWDGE (single
semaphore, cumulative ticks), evictions per psum-slot stay on a single
engine, and a few tiny "observer" ops (ldweights / 1-elem copies) absorb
a second wait where structurally unavoidable.  _fix_sync_waits hoists any
remaining excess waits onto NoOps.
"""

import sys

import numpy as np
import ml_dtypes

sys.path.insert(0, "/opt/trn_rl_repo")

B, N, C = 2, 2048, 1024
H, D = 16, 64
SCALE = D**-0.5
NCORES = 8
HPC = H // NCORES  # heads per core = 2

bf16 = ml_dtypes.bfloat16

_graph_cache = {}


def _fix_sync_waits(nc):
    """Walrus in this toolchain accepts at most ONE sync wait on compute
    instructions (two on DMAs).  Tile emits more.  Fix up the built graph:
      - drop waits on the instruction's own scheduled proc (in-order
        execution of a proc makes them always-satisfied),
      - hoist remaining excess waits onto standalone NoOps inserted just
        before the instruction on the same engine.
    """
    from concourse import mybir
    from concourse.tile_sem_assignment import PROC_NAME_TO_IDX

    idx_to_proc = {v: k for k, v in PROC_NAME_TO_IDX.items()}
    fixid = [0]
    for fn in nc.m.functions:
        for bb in fn.blocks:
            insts = list(bb.instructions)
            out = []
            changed = False
            for inst in insts:
                si = inst.sync_info
                if not si or not si.on_wait:
                    out.append(inst)
                    continue
                own = idx_to_proc.get(inst.bass_scheduled_proc, None)
                waits = list(si.on_wait)
                if own is not None:
                    kept = [
                        w
                        for w in waits
                        if w.ant_name.rsplit("_", 1)[0] != own
                    ]
                else:
                    kept = waits
                limit = 1
                hoist = []
                if len(kept) > limit:
                    hoist = kept[: len(kept) - limit]
                    kept = kept[len(kept) - limit :]
                if len(kept) != len(waits) or hoist:
                    changed = True
                    for w in hoist:
                        fixid[0] += 1
                        nop = mybir.InstNoOp(
                            name=f"W-fix-{fixid[0]}",
                            ins=[],
                            outs=[],
                            engine=inst.engine,
                            bass_nofuse=True,
                            text_hint="wait-split",
                            sync_info=mybir.SyncInfo(on_wait=[w], on_update=[]),
                        )
                        out.append(nop)
                    si.on_wait = kept
                out.append(inst)
            if changed:
                bb.instructions = out
    return nc


def _build_graph():
    import concourse.bass as bass
    import concourse.tile as tile
    from concourse import mybir

    EXP = mybir.ActivationFunctionType.Exp
    fp32 = mybir.dt.float32
    bfl = mybir.dt.bfloat16

    nc = bass.Bass()

    xt_d = nc.declare_dram_parameter("xt", [B, 8, 128, N], bfl, isOutput=False)
    wq_d = nc.declare_dram_parameter("wq", [128, 8, 128], bfl, isOutput=False)
    wk_d = nc.declare_dram_parameter("wk", [128, 8, 128], bfl, isOutput=False)
    wv_d = nc.declare_dram_parameter("wv", [128, 8, 128], bfl, isOutput=False)
    bq_d = nc.declare_dram_parameter("bq", [128, 1], fp32, isOutput=False)
    bv_d = nc.declare_dram_parameter("bv", [128, 128], fp32, isOutput=False)
    eb_d = nc.declare_dram_parameter(
        "expb", [2, 16, 128, HPC * 1024], bfl, isOutput=False
    )
    pw_d = nc.declare_dram_parameter("pw", [128, 1024], bfl, isOutput=False)
    out_d = nc.declare_dram_parameter("out", [B, 16, 128, C], bfl, isOutput=True)
    dbg = {}
    if _graph_cache.get("debug"):
        dbg["qt"] = nc.declare_dram_parameter("dbg_qt", [128, N], bfl, isOutput=True)
        dbg["kt"] = nc.declare_dram_parameter("dbg_kt", [128, N], bfl, isOutput=True)
        dbg["va"] = nc.declare_dram_parameter("dbg_va", [128, 16, 65], bfl, isOutput=True)
        dbg["aot"] = nc.declare_dram_parameter("dbg_aot", [128, N], bfl, isOutput=True)
        dbg["rec"] = nc.declare_dram_parameter("dbg_rec", [128, N], fp32, isOutput=True)
        dbg["es"] = nc.declare_dram_parameter("dbg_es", [128, 2048], bfl, isOutput=True)

    with tile.TileContext(nc) as tc:
        with (
            tc.tile_pool(name="weights", bufs=1) as wpool,
            tc.tile_pool(name="xt", bufs=1) as xpool,
            tc.tile_pool(name="qkvt", bufs=1) as qkpool,
            tc.tile_pool(name="eb", bufs=4) as ebpool,
            tc.tile_pool(name="es", bufs=6) as espool,
            tc.tile_pool(name="norm", bufs=2) as rpool,
            tc.tile_pool(name="outsb", bufs=4) as opool,
            tc.tile_pool(name="scratch", bufs=1) as spool,
            tc.tile_pool(name="psbig", bufs=2, space="PSUM") as psbig,
            tc.tile_pool(name="psacc", bufs=1, space="PSUM") as psacc,
            tc.tile_pool(name="dramsc", bufs=2, space="DRAM") as dpool,
        ):
            # ---- persistent weights, all on SWDGE ----
            wq = wpool.tile([128, 8, 128], bfl)
            wk = wpool.tile([128, 8, 128], bfl)
            wv = wpool.tile([128, 8, 128], bfl)
            pw = wpool.tile([128, 1024], bfl)
            bq = wpool.tile([128, 1], fp32)
            bv = wpool.tile([128, 128], fp32)
            nc.sync.dma_start(out=wq[:], in_=wq_d[:])
            nc.sync.dma_start(out=wk[:], in_=wk_d[:])
            nc.sync.dma_start(out=wv[:], in_=wv_d[:])
            nc.sync.dma_start(out=pw[:], in_=pw_d[:])
            nc.sync.dma_start(out=bq[:], in_=bq_d[:])
            nc.sync.dma_start(out=bv[:], in_=bv_d[:])
            # observers: DVE syncs on the bias DMAs once, so later evictions
            # carry only their PE wait
            warm = spool.tile([128, 128], fp32)
            nc.vector.tensor_copy(out=warm[:, 0:1], in_=bq[:])
            nc.vector.tensor_copy(out=warm[:], in_=bv[:])
            act_scr = spool.tile([1, 8], bfl)
            dve_scr = spool.tile([1, 8], bfl)

            # ---- all x tiles for BOTH batches, prefetched up front ----
            xts = {}
            for b in range(B):
                for ci in range(8):
                    xtile = xpool.tile(
                        [128, N], bfl, tag=f"xt{b}_{ci}", name=f"xt_{b}_{ci}"
                    )
                    nc.sync.dma_start(out=xtile[:], in_=xt_d[b, ci])
                    xts[(b, ci)] = xtile

            qt = [
                qkpool.tile([128, N], bfl, tag=f"qt{b}", name=f"qt{b}")
                for b in range(B)
            ]
            kt = [
                qkpool.tile([128, N], bfl, tag=f"kt{b}", name=f"kt{b}")
                for b in range(B)
            ]
            aot = [
                qkpool.tile([128, N], bfl, tag=f"aot{b}", name=f"aot{b}")
                for b in range(B)
            ]
            # v augmented with ones column: [m-part, mi, 65]
            vaug = [
                qkpool.tile([128, 16, 65], bfl, tag=f"vaug{u}", name=f"vaug{u}")
                for u in range(B * HPC)
            ]
            for u in range(B * HPC):
                nc.vector.memset(vaug[u][:, :, 64:65], 1.0)

            # softmax reciprocal, bf16, broadcast across head-dim partitions;
            # applied to raw attention outputs once, just before projection
            rec2 = [
                qkpool.tile([128, N], fp32, tag=f"rec2{b}", name=f"rec2{b}")
                for b in range(B)
            ]

            # ---- stage helpers (call order == engine issue order) ----

            def qk_tile(b, ni):
                """q^T,k^T for one 512-col n-chunk: [128 j, 512] each."""
                nsl = slice(512 * ni, 512 * ni + 512)
                psq = psbig.tile([128, 1024], fp32, tag="ps")
                for ci in range(8):
                    nc.tensor.matmul(
                        psq[:, 0:512], lhsT=wq[:, ci, :], rhs=xts[(b, ci)][:, nsl],
                        start=(ci == 0), stop=(ci == 7),
                    )
                    nc.tensor.matmul(
                        psq[:, 512:1024], lhsT=wk[:, ci, :], rhs=xts[(b, ci)][:, nsl],
                        start=(ci == 0), stop=(ci == 7),
                    )
                nc.vector.tensor_scalar_add(
                    out=qt[b][:, nsl], in0=psq[:, 0:512], scalar1=bq[:]
                )
                nc.vector.tensor_copy(out=kt[b][:, nsl], in_=psq[:, 512:1024])

            def v_chunk(b, si):
                """v rows [128 m] for one si chunk, both heads."""
                msl = slice(128 * si, 128 * si + 128)
                psv = psbig.tile([128, 1024], fp32, tag="ps")
                for ci in range(8):
                    nc.tensor.matmul(
                        psv[:, 0:128], lhsT=xts[(b, ci)][:, msl], rhs=wv[:, ci, :],
                        start=(ci == 0), stop=(ci == 7),
                    )
                for hi in range(HPC):
                    jsl = slice(64 * hi, 64 * hi + 64)
                    nc.vector.tensor_add(
                        out=vaug[b * HPC + hi][:, si, 0:64],
                        in0=psv[:, jsl], in1=bv[:, jsl],
                    )

            def b_block(b, ni, inject=None):
                """16 software-pipelined attention iterations + drain.

                inject: {mi: [callable, ...]} — extra work (stage A/C chunks)
                issued between scores(mi) and PV(mi-1).  The injected matmuls
                fill the PE-idle part of each ACT-bound iteration and keep
                the HAM clock gate warm."""
                inject = inject or {}
                nsl = slice(1024 * ni, 1024 * ni + 1024)
                pvA = psacc.tile([65, 1024], fp32, tag="pvA")
                pvB = psacc.tile([65, 1024], fp32, tag="pvB")

                def scores(mi):
                    msl = slice(128 * mi, 128 * mi + 128)
                    ps = psbig.tile([128, 1024], fp32, tag="ps", name=f"psX_{b}_{ni}_{mi}")
                    ps2 = psbig.tile([128, 1024], fp32, tag="ps", name=f"psY_{b}_{ni}_{mi}")
                    for half in range(2):
                        fsl = slice(512 * half, 512 * half + 512)
                        qsl = slice(
                            1024 * ni + 512 * half, 1024 * ni + 512 * half + 512
                        )
                        nc.tensor.matmul(
                            ps[:, fsl], lhsT=kt[b][0:64, msl],
                            rhs=qt[b][0:64, qsl], start=True, stop=True,
                        )
                        nc.tensor.matmul(
                            ps2[:, fsl], lhsT=kt[b][64:128, msl],
                            rhs=qt[b][64:128, qsl], start=True, stop=True,
                        )
                    return ps, ps2

                def pv(mi, es2):
                    for half in range(2):
                        fsl = slice(512 * half, 512 * half + 512)
                        f2 = slice(1024 + 512 * half, 1024 + 512 * half + 512)
                        nc.tensor.matmul(
                            pvA[0:65, fsl], lhsT=vaug[b * HPC][:, mi, :],
                            rhs=es2[:, fsl],
                            start=(mi == 0), stop=(mi == 15),
                        )
                        nc.tensor.matmul(
                            pvB[0:65, fsl], lhsT=vaug[b * HPC + 1][:, mi, :],
                            rhs=es2[:, f2],
                            start=(mi == 0), stop=(mi == 15),
                        )

                pending = None  # (mi, es2) awaiting PV issue
                for mi in range(16):
                    eb2 = ebpool.tile([128, HPC * 1024], bfl)
                    nc.sync.dma_start(out=eb2[:], in_=eb_d[ni, mi])
                    psX, psY = scores(mi)
                    for fn in inject.get(mi, ()):
                        fn()
                    if pending is not None:
                        pv(*pending)
                    # es2 holds both heads side by side: [128, 2048]
                    es2 = espool.tile([128, 2 * 1024], bfl)
                    nc.scalar.activation(out=es2[:, 0:1024], in_=psX[:], func=EXP)
                    nc.scalar.activation(out=es2[:, 1024:2048], in_=psY[:], func=EXP)
                    # DVE observer: absorb the eb2 DMA wait
                    nc.vector.tensor_copy(out=dve_scr[0:1, 0:1], in_=eb2[0:1, 0:1])
                    nc.vector.tensor_mul(out=es2[:], in0=es2[:], in1=eb2[:])
                    if dbg and b == 0 and ni == 0 and mi == 0:
                        nc.sync.dma_start(out=dbg["es"][:], in_=es2[:])
                    pending = (mi, es2)
                pv(*pending)

                # drain the PV psums: raw outputs + the sums row, then the
                # reciprocal chain on a [128, 8] relayout via DRAM bounce
                for hi, pvt in ((0, pvA), (1, pvB)):
                    hs = slice(64 * hi, 64 * hi + 64)
                    dn = rpool.tile([1, 1024], fp32, tag=f"dn{hi}")
                    if hi == 0:
                        nc.scalar.copy(out=dn[:], in_=pvt[64:65, :])
                        nc.scalar.copy(out=aot[b][hs, nsl], in_=pvt[0:64, :])
                    else:
                        nc.vector.tensor_copy(out=dn[:], in_=pvt[64:65, :])
                        nc.vector.tensor_copy(
                            out=aot[b][hs, nsl], in_=pvt[0:64, :]
                        )
                    d1 = dpool.tile([1, 1024], fp32, tag=f"d1{hi}")
                    nc.sync.dma_start(out=d1[:], in_=dn[:])
                    dc = rpool.tile([128, 8], fp32, tag=f"dc{hi}")
                    d1_col = bass.AP(
                        tensor=d1.tensor, offset=d1.offset,
                        ap=[[8, 128], [1, 8]],
                    )
                    nc.sync.dma_start(out=dc[:], in_=d1_col)
                    rcol = rpool.tile([128, 8], fp32, tag=f"rc{hi}")
                    nc.vector.reciprocal(out=rcol[:], in_=dc[:])
                    d2 = dpool.tile([1, 1024], fp32, tag=f"d2{hi}")
                    d2_col = bass.AP(
                        tensor=d2.tensor, offset=d2.offset,
                        ap=[[8, 128], [1, 8]],
                    )
                    nc.sync.dma_start(out=d2_col, in_=rcol[:])
                    d2_bcast = bass.AP(
                        tensor=d2.tensor, offset=d2.offset,
                        ap=[[0, 64]] + list(d2.ap),
                    )
                    nc.sync.dma_start(out=rec2[b][hs, nsl], in_=d2_bcast)

            def c_norm(b):
                """apply the deferred softmax normalization in one pass"""
                if dbg and b == 0:
                    nc.sync.dma_start(out=dbg["qt"][:], in_=qt[0][:])
                    nc.sync.dma_start(out=dbg["kt"][:], in_=kt[0][:])
                    nc.sync.dma_start(out=dbg["va"][:], in_=vaug[0][:])
                    nc.sync.dma_start(out=dbg["rec"][:], in_=rec2[0][:])
                nc.vector.tensor_mul(out=aot[b][:], in0=aot[b][:], in1=rec2[b][:])
                if dbg and b == 0:
                    nc.sync.dma_start(out=dbg["aot"][:], in_=aot[0][:])
                # PE observer: absorb aot's fresh DVE tick
                nc.tensor.ldweights(weights=aot[b][0:1, 0:1])

            def c_chunk(b, si, evict=None):
                msl = slice(128 * si, 128 * si + 128)
                psp = psbig.tile([128, 1024], fp32, tag="ps")
                for half in range(2):
                    fsl = slice(512 * half, 512 * half + 512)
                    nc.tensor.matmul(
                        psp[:, fsl], lhsT=aot[b][:, msl], rhs=pw[:, fsl],
                        start=True, stop=True,
                    )
                ob = opool.tile([128, 1024], bfl)
                # evictions alternate engines by si parity (or forced via
                # evict=); a tiny write first absorbs the out-DMA's WAR tick
                if evict is None:
                    evict = "act" if si % 2 == 0 else "dve"
                if evict == "act":
                    nc.scalar.copy(out=ob[0:1, 0:1], in_=act_scr[0:1, 0:1])
                    nc.scalar.copy(out=ob[:], in_=psp[:])
                else:
                    nc.vector.memset(ob[0:1, 0:1], 0.0)
                    nc.vector.tensor_copy(out=ob[:], in_=psp[:])
                nc.sync.dma_start(out=out_d[b, si], in_=ob[:])

            # ---- schedule: minimal prefix, then stage A/C chunks injected
            # into the ACT-bound attention blocks ----
            qk_tile(0, 0)
            qk_tile(0, 1)
            v_chunk(0, 0)
            v_chunk(0, 1)
            # ACT observer: sync ACT on DVE once before first exps
            nc.scalar.copy(
                out=act_scr[0:1, 0:1], in_=vaug[HPC - 1][0:1, 1, 0:1]
            )
            inj00 = {mi: [lambda si=mi + 2: v_chunk(0, si)] for mi in range(14)}
            inj00[3] = inj00[3] + [lambda: qk_tile(0, 2)]
            inj00[7] = inj00[7] + [lambda: qk_tile(0, 3)]
            b_block(0, 0, inj00)
            inj01 = {
                2: [lambda: qk_tile(1, 0)],
                6: [lambda: qk_tile(1, 1)],
                10: [lambda: v_chunk(1, 0)],
                13: [lambda: v_chunk(1, 1)],
            }
            b_block(0, 1, inj01)
            inj10 = {mi: [lambda si=mi + 2: v_chunk(1, si)] for mi in range(14)}
            inj10[3] = inj10[3] + [lambda: qk_tile(1, 2)]
            inj10[7] = inj10[7] + [lambda: qk_tile(1, 3)]
            b_block(1, 0, inj10)
            inj11 = {1: [lambda: c_norm(0)]}
            for idx, mi in enumerate((3, 6, 9, 12, 15)):
                inj11[mi] = [lambda si=idx: c_chunk(0, si, evict="dve")]
            b_block(1, 1, inj11)
            for si in range(5, 16):
                c_chunk(0, si)
            c_norm(1)
            for si in range(16):
                c_chunk(1, si)

    _fix_sync_waits(nc)
    return nc


def _prep_inputs(x, rel_pos_bias, qkv_w, q_bias, v_bias):
    """Build the 8 per-core input maps (host-side shard + transpose + cast)."""
    x = np.asarray(x, dtype=np.float32)
    rel_pos_bias = np.asarray(rel_pos_bias, dtype=np.float32)
    qkv_w = np.asarray(qkv_w, dtype=np.float32)
    q_bias = np.asarray(q_bias, dtype=np.float32)
    v_bias = np.asarray(v_bias, dtype=np.float32)

    # xT: [b, c, n] -> [b, 8, 128, n]
    xt = np.ascontiguousarray(x.transpose(0, 2, 1)).reshape(B, 8, 128, N).astype(bf16)

    in_maps = []
    for c in range(NCORES):
        heads = [HPC * c + i for i in range(HPC)]
        jrows = np.concatenate([np.arange(64 * h, 64 * h + 64) for h in heads])

        def tile_w(rows, scale=1.0):
            wt = (scale * qkv_w[rows]).T.astype(bf16)  # [1024 c, 128 j]
            return np.ascontiguousarray(wt.reshape(8, 128, 128).transpose(1, 0, 2))

        wq = tile_w(jrows, SCALE)
        wk = tile_w(C + jrows)
        wv = tile_w(2 * C + jrows)
        bq = (SCALE * q_bias[jrows]).reshape(128, 1).astype(np.float32)
        bv = np.ascontiguousarray(
            np.broadcast_to(v_bias[jrows][None, :], (128, 128)).astype(np.float32)
        )
        # exp of transposed bias, tiled: [ni, mi, 128 m, hpc, 1024 n]
        ebt = np.exp(rel_pos_bias[heads].transpose(0, 2, 1))  # [hpc, m, n]
        ebt = np.ascontiguousarray(
            ebt.reshape(HPC, 16, 128, 2, 1024).transpose(3, 1, 2, 0, 4)
        ).astype(bf16).reshape(2, 16, 128, HPC * 1024)
        in_maps.append(
            {"xt": xt, "wq": wq, "wk": wk, "wv": wv, "bq": bq, "bv": bv, "expb": ebt}
        )
    return in_maps


def kernel(x, rel_pos_bias, qkv_w, q_bias, v_bias, proj_w, proj_b):
    from concourse.bass_utils import run_bass_kernel_spmd

    x = np.asarray(x, dtype=np.float32)
    proj_w = np.asarray(proj_w, dtype=np.float32)
    proj_b = np.asarray(proj_b, dtype=np.float32)

    if "nc" not in _graph_cache:
        _graph_cache["nc"] = _build_graph()
    nc = _graph_cache["nc"]

    in_maps = _prep_inputs(x, rel_pos_bias, qkv_w, q_bias, v_bias)
    for c in range(NCORES):
        heads = [HPC * c + i for i in range(HPC)]
        jrows = np.concatenate([np.arange(64 * h, 64 * h + 64) for h in heads])
        pw = np.ascontiguousarray(proj_w[:, jrows].T.astype(bf16))  # [128 j, 1024 e]
        in_maps[c]["pw"] = pw

    res = run_bass_kernel_spmd(nc, in_maps, core_ids=list(range(NCORES)))
    out = np.zeros((B, 16, 128, C), dtype=np.float32)
    for r in res.results:
        out += np.asarray(r["out"], dtype=np.float32)
    out = out.reshape(B, N, C) + proj_b[None, None, :]
    return out
